# revision 24
# baseline (speedup 1.0000x reference)
"""MoE HTR FeedForward kernel for Trainium2 (8 NeuronCores, data-parallel over nodes).

Self-contained: takes full inputs, shards over 8 cores internally, returns full output.

v3: scatter-mean emits cT directly (tt-stationary matmuls; t rows pre-scaled
by recip[dst] on the host so the one-hot stays a single fast is_equal),
PSUM rebalanced so phase A runs two block accumulators in parallel.
"""
import sys

try:
    import concourse.bass as bass  # noqa: F401
except ImportError:  # pragma: no cover
    sys.path.insert(0, "/opt/trn_rl_repo")

import numpy as np
import ml_dtypes

import concourse.bass as bass
import concourse.bacc as bacc
import concourse.tile as tile
import concourse.mybir as mybir
from concourse.bass_utils import run_bass_kernel_spmd

FP32 = mybir.dt.float32
BF16 = mybir.dt.bfloat16
AF = mybir.ActivationFunctionType
OP = mybir.AluOpType
BNP = ml_dtypes.bfloat16

# Problem constants (hardcoded per the harness contract).
N_NODES = 8192
N_EDGES = 262144
C = 128          # sphere channels
H = 512          # hidden channels
EC = 128         # edge channels
K = 4            # experts (2 equivariant + 2 non-equivariant)
LMAX = 4
M = 25           # spherical coeffs
N_CORES = 8
NPC = N_NODES // N_CORES       # 1024 nodes per core
SBLK = 8                       # scatter blocks of 128 nodes per core
NBLK = 2                       # node blocks of 512 for the main GEMMs
NB = NPC // NBLK               # 512
LM = [0] + [1] * 3 + [2] * 5 + [3] * 7 + [4] * 9  # m -> l


def build_program(t_b: int, skew=6, act_copy=None, repeat=1,
                  hp_bufs=4, oh_gps_mod=0) -> bass.Bass:
    """Build the per-core Bass program. t_b = edge tiles (of 128) per 128-node block.

    oh_gps_mod: every oh_gps_mod-th one-hot build goes to GpSimd (0 = none).
    """
    e_blk = t_b * 128            # padded edges per scatter block
    n_tiles = SBLK * t_b         # edge tiles per core

    nc = bacc.Bacc("TRN2", target_bir_lowering=False, debug=False)

    xt = nc.declare_dram_parameter("xt", [128, M * NPC], BF16, isOutput=False)
    tpad = nc.declare_dram_parameter("tpad", [128, n_tiles * EC], BF16, isOutput=False)
    dstcol = nc.declare_dram_parameter("dstcol", [128, n_tiles], FP32, isOutput=False)
    w1d = nc.declare_dram_parameter("w1", [128, K * H], BF16, isOutput=False)
    w2d = nc.declare_dram_parameter("w2", [128, K * H], BF16, isOutput=False)
    gwed = nc.declare_dram_parameter("gwe", [128, 2 * 2560], BF16, isOutput=False)
    gbed = nc.declare_dram_parameter("gbe", [128, 2 * 20], FP32, isOutput=False)
    gwnd = nc.declare_dram_parameter("gwn", [128, 2 * H], BF16, isOutput=False)
    gbnd = nc.declare_dram_parameter("gbn", [128, 2 * 4], FP32, isOutput=False)
    b1nd = nc.declare_dram_parameter("b1n", [128, 2 * 4], FP32, isOutput=False)
    b2nd = nc.declare_dram_parameter("b2n", [2, 128], BF16, isOutput=False)
    rw1d = nc.declare_dram_parameter("rw1", [128, EC], BF16, isOutput=False)
    rb1d = nc.declare_dram_parameter("rb1", [128, 1], FP32, isOutput=False)
    rw2d = nc.declare_dram_parameter("rw2", [128, K], BF16, isOutput=False)
    rb2d = nc.declare_dram_parameter("rb2", [4, 1], FP32, isOutput=False)
    eseld = nc.declare_dram_parameter("esel", [4, K * 128], BF16, isOutput=False)
    outd = nc.declare_dram_parameter("out", [128, M * NPC], BF16, isOutput=True)

    with tile.TileContext(nc) as tc:
        with tc.tile_pool(name="persist", bufs=1) as pp:
            # --- persistent SBUF tiles ---
            iota_t = pp.tile([128, 128], BF16)
            nc.gpsimd.iota(iota_t[:], [[1, 128]], channel_multiplier=0,
                           allow_small_or_imprecise_dtypes=True)

            dst_sb = pp.tile([128, n_tiles], FP32)
            nc.sync.dma_start(dst_sb[:], dstcol[:])

            w1_sb = pp.tile([128, K * H], BF16)
            nc.scalar.dma_start(w1_sb[:], w1d[:])
            w2_sb = pp.tile([128, K * H], BF16)
            nc.scalar.dma_start(w2_sb[:], w2d[:])
            gwe_sb = pp.tile([128, 2 * 2560], BF16)
            nc.scalar.dma_start(gwe_sb[:], gwed[:])
            gbe_sb = pp.tile([128, 2 * 20], FP32)
            nc.scalar.dma_start(gbe_sb[:], gbed[:])
            gwn_sb = pp.tile([128, 2 * H], BF16)
            nc.scalar.dma_start(gwn_sb[:], gwnd[:])
            gbn_sb = pp.tile([128, 2 * 4], FP32)
            nc.scalar.dma_start(gbn_sb[:], gbnd[:])
            b1n_sb = pp.tile([128, 2 * 4], FP32)
            nc.scalar.dma_start(b1n_sb[:], b1nd[:])
            b2n_sb = pp.tile([2, 128], BF16)
            nc.scalar.dma_start(b2n_sb[:], b2nd[:])
            rw1_sb = pp.tile([128, EC], BF16)
            nc.scalar.dma_start(rw1_sb[:], rw1d[:])
            rb1_sb = pp.tile([128, 1], FP32)
            nc.scalar.dma_start(rb1_sb[:], rb1d[:])
            rw2_sb = pp.tile([128, K], BF16)
            nc.scalar.dma_start(rw2_sb[:], rw2d[:])
            rb2_sb = pp.tile([4, 1], FP32)
            nc.scalar.dma_start(rb2_sb[:], rb2d[:])
            esel_sb = pp.tile([4, K * 128], BF16)
            nc.scalar.dma_start(esel_sb[:], eseld[:])

            cT_h = pp.tile([128, NPC], BF16)       # c transposed: [EC, n], fp16
            wT_bf = pp.tile([4, NPC], BF16)        # router weights transposed
            wTne_bf = pp.tile([2, NPC], BF16)      # ne-expert rows of w
            wb_sb = pp.tile([128, K * NPC], BF16)  # router weights broadcast per expert
            ge_sb = pp.tile([128, 40 * NPC], BF16)  # eq gates: (e*20+gt) tiles of [128, NPC]
            gn_sb = pp.tile([128, 8 * NPC], BF16)   # ne gates: (e*4+ht)

            # body emitted `repeat` times (>1 only for timing runs)
            def emit_all(rep):
                # ---- Phase A: scatter-mean -> cT_h directly.  Per block:
                # oh = (iota == dst) one-hot; t rows are pre-scaled by
                # recip[dst] on the host, so cT block = sum_ti tt_ti.T @ oh_ti
                # accumulated in PSUM is already the mean.
                def phase_a(jh, pa_ps, pa_oh, pa_t):
                    for b in range(4 * jh, 4 * jh + 4):
                        tt = pa_t.tile([128, t_b * EC], BF16, tag="t")
                        nc.sync.dma_start(
                            tt[:], tpad[:, b * t_b * EC:(b + 1) * t_b * EC])
                        cps = pa_ps.tile([128, 128], FP32, tag="s")
                        for ti in range(t_b):
                            col = b * t_b + ti
                            oh = pa_oh.tile([128, 128], BF16, tag="oh")
                            eng = nc.gpsimd if (
                                oh_gps_mod and ti % oh_gps_mod == oh_gps_mod - 1
                            ) else nc.vector
                            eng.tensor_scalar(
                                oh[:], iota_t[:],
                                dst_sb[:, col:col + 1], None, op0=OP.is_equal)
                            nc.tensor.matmul(
                                cps[:], tt[:, ti * EC:(ti + 1) * EC], oh[:],
                                start=(ti == 0), stop=(ti == t_b - 1))
                        nc.vector.tensor_copy(cT_h[:, b * 128:(b + 1) * 128], cps[:])

                ones4 = pp.tile([4, 1], FP32)
                nc.gpsimd.memset(ones4[:], 1.0)
                ones1 = pp.tile([1, 4], FP32)
                nc.gpsimd.memset(ones1[:], 1.0)

                def phase_b(j):
                    # softmax without transposes: Exp on [4, NB], partition-dim
                    # sum and broadcast via tiny matmuls.
                    with tc.tile_pool(name=f"pb{rep}{j}_ps", bufs=1, space="PSUM") as pb_ps, \
                         tc.tile_pool(name=f"pb{rep}{j}_sb", bufs=2) as pb_sb:
                        z1p = pb_ps.tile([128, NB], FP32, tag="ps")
                        nc.tensor.matmul(z1p[:], rw1_sb[:], cT_h[:, j * NB:(j + 1) * NB])
                        z1s = pb_sb.tile([128, NB], BF16, tag="z1")
                        nc.scalar.activation(z1s[:], z1p[:], AF.Silu, bias=rb1_sb[:, 0:1])
                        z2p = pb_ps.tile([4, NB], FP32, tag="ps")
                        nc.tensor.matmul(z2p[:], rw2_sb[:], z1s[:])
                        ex = pb_sb.tile([4, NB], FP32, tag="ex")
                        nc.scalar.activation(ex[:], z2p[:], AF.Exp, bias=rb2_sb[:, 0:1])
                        smp = pb_ps.tile([1, NB], FP32, tag="ps")
                        nc.tensor.matmul(smp[:], ones4[:], ex[:])
                        rcp = pb_sb.tile([1, NB], FP32, tag="rc")
                        nc.vector.reciprocal(rcp[:], smp[:])
                        bcp = pb_ps.tile([4, NB], FP32, tag="ps")
                        nc.tensor.matmul(bcp[:], ones1[:], rcp[:])
                        nc.vector.tensor_tensor(
                            wT_bf[:, j * NB:(j + 1) * NB], ex[:], bcp[:], op=OP.mult)
                        wnp = pb_ps.tile([2, NB], FP32, tag="ps")
                        nc.tensor.matmul(wnp[:], esel_sb[:4, 256:512:128],
                                         wT_bf[:4, j * NB:(j + 1) * NB])
                        nc.vector.tensor_copy(wTne_bf[:, j * NB:(j + 1) * NB], wnp[:])
                        for e in range(K):
                            wbp = pb_ps.tile([128, NB], FP32, tag="ps")
                            nc.tensor.matmul(
                                wbp[:], esel_sb[:, e * 128:(e + 1) * 128],
                                wT_bf[:4, j * NB:(j + 1) * NB])
                            nc.vector.tensor_copy(
                                wb_sb[:, e * NPC + j * NB:e * NPC + (j + 1) * NB],
                                wbp[:])

                # ------- Phases C+D: one software-pipelined unit stream -------
                # Units: ("g", e, gt, j) = gate tile; ("m", m, j, e, ht) = main
                # expert tile. Front stream: GEMM1 into the shared "hp" PSUM
                # pool + SiLU (frees the PSUM slot early). Back stream (skewed):
                # gate multiply + GEMM2. These pools open FIRST so they own the
                # low PSUM banks; phases A/B nest inside using the remaining
                # banks — the second node-block half of A/B then overlaps the
                # first block's main pipeline.
                with tc.tile_pool(name=f"cd_hp{rep}", bufs=hp_bufs, space="PSUM") as hp_pool, \
                     tc.tile_pool(name=f"cd_op{rep}", bufs=2, space="PSUM") as op_pool, \
                     tc.tile_pool(name=f"cd_gs{rep}", bufs=8) as gs_pool, \
                     tc.tile_pool(name=f"cd_xs{rep}", bufs=3) as xs_pool, \
                     tc.tile_pool(name=f"cd_hs{rep}", bufs=12) as hs_pool, \
                     tc.tile_pool(name=f"cd_tmp{rep}", bufs=12) as tmp_pool, \
                     tc.tile_pool(name=f"cd_os{rep}", bufs=3) as os_pool:
                    SKEW = skew
                    ACT_COPY_SLOTS = set(act_copy) if act_copy is not None \
                        else {(0, 0), (1, 0)}
                    EQ8 = [(0, 0), (0, 1), (0, 2), (0, 3),
                           (1, 0), (1, 1), (1, 2), (1, 3)]
                    NE_ALL = [(e, ht) for e in (2, 3) for ht in range(4)]
                    FRONT_ORDER = EQ8 + NE_ALL
                    BACK_ORDER = EQ8 + NE_ALL

                    front_units = []
                    back_units = []
                    # j-major: the whole second node-block (router, gates,
                    # mains) trails the first, hiding its preamble. m=0
                    # (ACT-heavy eq SiLU) sits mid-pass; gate batches are split
                    # in half and interleaved sparsely into the two preceding
                    # m-iterations; j=1's initial gates hide late in j=0.
                    M_ORDER = [1, 2, 3] + [0] + list(range(4, M))
                    GATE_BEFORE = {0: 0, 4: 2, 9: 3, 16: 4}

                    def gate_batch(l, j):
                        return [("g", e, l * 4 + ht, j) for e in (0, 1)
                                for ht in range(4)]

                    def interleave(mains, gates):
                        out = []
                        for i, u in enumerate(mains):
                            out.append(u)
                            if i % 4 == 1 and gates:
                                out.append(gates.pop(0))
                        out.extend(gates)
                        return out

                    NIT = len(M_ORDER)
                    for j in range(NBLK):
                        pend = [[] for _ in range(NIT)]
                        for m_t, l in GATE_BEFORE.items():
                            p = M_ORDER.index(m_t)
                            batch = gate_batch(l, j)
                            pend[max(0, p - 2)].extend(batch[:4])
                            pend[max(0, p - 1)].extend(batch[4:])
                        if j == 0:
                            init = [("g", e, ht, j) for e in (2, 3)
                                    for ht in range(4)]
                            init += gate_batch(1, j)
                            front_units.extend(init)
                            back_units.extend(init)
                            # j=1's initial gates ride along late in j=0
                            init1 = [("g", e, ht, 1) for e in (2, 3)
                                     for ht in range(4)] + gate_batch(1, 1)
                            for k, gu in enumerate(init1):
                                pend[17 + k // 4].append(gu)
                        for idx, m in enumerate(M_ORDER):
                            fr = [("m", m, j, e, ht) for (e, ht) in FRONT_ORDER]
                            bk = [("m", m, j, e, ht) for (e, ht) in BACK_ORDER]
                            front_units.extend(interleave(fr, list(pend[idx])))
                            back_units.extend(interleave(bk, list(pend[idx])))

                    u_hp = {}
                    u_tmp = {}
                    state = {}
                    back_cnt = {}
                    # first two m-iterations of j=0: force the ACT-copy path on
                    # all eq slots so hp PSUM slots free early while the back
                    # stream is still blocked on phase A/B gates.
                    EARLY_M = {(m, 0) for m in M_ORDER[:3]}

                    def use_act_copy(m, j, e, ht):
                        if (m, j) in EARLY_M:
                            return True
                        return (e, ht) in ACT_COPY_SLOTS

                    def emit_front(u):
                        if u[0] == "g":
                            _, e, gt, j = u
                            if e < 2:
                                wsrc = gwe_sb[:, (e * 20 + gt) * 128:
                                              (e * 20 + gt + 1) * 128]
                                bias = gbe_sb[:, e * 20 + gt:e * 20 + gt + 1]
                            else:
                                wsrc = gwn_sb[:, ((e - 2) * 4 + gt) * 128:
                                              ((e - 2) * 4 + gt + 1) * 128]
                                bias = gbn_sb[:, (e - 2) * 4 + gt:(e - 2) * 4 + gt + 1]
                            hp = hp_pool.tile([128, NB], FP32, tag="hp")
                            nc.tensor.matmul(hp[:], wsrc, cT_h[:, j * NB:(j + 1) * NB])
                            gs = gs_pool.tile([128, NB], BF16, tag="gs")
                            nc.scalar.activation(gs[:], hp[:], AF.Silu, bias=bias)
                            u_tmp[u] = gs
                            return
                        _, m, j, e, ht = u
                        if (e, ht) == FRONT_ORDER[0]:
                            xs = xs_pool.tile([128, NB], BF16, tag="xs")
                            nc.sync.dma_start(
                                xs[:], xt[:, m * NPC + j * NB:m * NPC + (j + 1) * NB])
                            state[("xs", m, j)] = xs
                        hp = hp_pool.tile([128, NB], FP32, tag="hp")
                        nc.tensor.matmul(
                            hp[:], w1_sb[:, e * H + ht * 128:e * H + (ht + 1) * 128],
                            state[("xs", m, j)][:])
                        if e >= 2:
                            tmp = tmp_pool.tile([128, NB], BF16, tag="tmp")
                            nc.scalar.activation(
                                tmp[:], hp[:], AF.Silu,
                                bias=b1n_sb[:, (e - 2) * 4 + ht:(e - 2) * 4 + ht + 1])
                            u_tmp[u] = tmp
                        elif m == 0:
                            tmp = tmp_pool.tile([128, NB], BF16, tag="tmp")
                            nc.scalar.activation(tmp[:], hp[:], AF.Silu)
                            u_tmp[u] = tmp
                        elif use_act_copy(m, j, e, ht):
                            tmp = tmp_pool.tile([128, NB], BF16, tag="tmp")
                            nc.scalar.copy(tmp[:], hp[:])
                            u_tmp[u] = tmp
                        else:
                            u_hp[u] = hp

                    def emit_back(u):
                        if u[0] == "g":
                            _, e, gt, j = u
                            gs = u_tmp.pop(u)
                            if e < 2:
                                col = (e * 20 + gt) * NPC + j * NB
                                dstt = ge_sb
                            else:
                                col = ((e - 2) * 4 + gt) * NPC + j * NB
                                dstt = gn_sb
                            nc.vector.tensor_tensor(
                                dstt[:, col:col + NB], gs[:],
                                wb_sb[:, e * NPC + j * NB:e * NPC + (j + 1) * NB],
                                op=OP.mult)
                            return
                        _, m, j, e, ht = u
                        key = (m, j)
                        if key not in back_cnt:
                            back_cnt[key] = 0
                            op = op_pool.tile([128, NB], FP32, tag="op")
                            state[("op",) + key] = op
                            # rank-2 bias term: sum_e w[n,2+e] * ne_b2[e,:]
                            nc.tensor.matmul(
                                op[:], b2n_sb[:2, :], wTne_bf[:2, j * NB:(j + 1) * NB],
                                start=True, stop=False)
                        op = state[("op",) + key]
                        back_cnt[key] += 1
                        last = back_cnt[key] == 16
                        hs = hs_pool.tile([128, NB], BF16, tag="hs")
                        if u in u_hp:
                            hp = u_hp.pop(u)
                            col = (e * 20 + LM[m] * 4 + ht) * NPC + j * NB
                            nc.vector.tensor_tensor(
                                hs[:], hp[:], ge_sb[:, col:col + NB], op=OP.mult)
                        else:
                            tmp = u_tmp.pop(u)
                            if e < 2:
                                col = (e * 20 + LM[m] * 4 + ht) * NPC + j * NB
                                gsl = ge_sb[:, col:col + NB]
                            else:
                                col = ((e - 2) * 4 + ht) * NPC + j * NB
                                gsl = gn_sb[:, col:col + NB]
                            nc.vector.tensor_tensor(hs[:], tmp[:], gsl, op=OP.mult)
                        nc.tensor.matmul(
                            op[:], w2_sb[:, e * H + ht * 128:e * H + (ht + 1) * 128],
                            hs[:], start=False, stop=last)
                        if last:
                            os = os_pool.tile([128, NB], BF16, tag="os")
                            nc.scalar.copy(os[:], op[:])
                            state.pop(("op",) + key)
                            nc.gpsimd.dma_start(
                                outd[:, m * NPC + j * NB:m * NPC + (j + 1) * NB], os[:])

                    # pre-emit the first 10 of m-group 1's fronts (GEMM1
                    # depends only on xt + w1, which land ~10us before the
                    # first scatter tile) so the PE stream has work at start.
                    # Capped below tmp_pool bufs (12): 16 deadlocks.
                    pre = [u for u in front_units
                           if u[0] == "m" and u[1] == M_ORDER[0] and u[2] == 0][:10]
                    for u in pre:
                        emit_front(u)
                    pre_done = set(pre)
                    with tc.tile_pool(name=f"pa{rep}0_ps", bufs=2, space="PSUM") as pa_ps, \
                         tc.tile_pool(name=f"pa{rep}0_oh", bufs=8) as pa_oh, \
                         tc.tile_pool(name=f"pa{rep}0_t", bufs=2) as pa_t:
                        phase_a(0, pa_ps, pa_oh, pa_t)
                    phase_b(0)
                    with tc.tile_pool(name=f"pa{rep}1_ps", bufs=2, space="PSUM") as pa_ps, \
                         tc.tile_pool(name=f"pa{rep}1_oh", bufs=8) as pa_oh, \
                         tc.tile_pool(name=f"pa{rep}1_t", bufs=2) as pa_t:
                        phase_a(1, pa_ps, pa_oh, pa_t)
                    phase_b(1)
                    fpos = {u: i for i, u in enumerate(front_units)}
                    req = max(fpos[u] - i for i, u in enumerate(back_units)) + 1
                    SKEW = max(SKEW, req)
                    T = len(front_units)
                    for t in range(T + SKEW):
                        if t < T and front_units[t] not in pre_done:
                            emit_front(front_units[t])
                        if t >= SKEW:
                            emit_back(back_units[t - SKEW])

            for _rep in range(repeat):
                emit_all(_rep)
    nc.compile()
    return nc


_PROGRAM_CACHE: dict = {}


def _get_program(t_b: int) -> bass.Bass:
    if t_b not in _PROGRAM_CACHE:
        _PROGRAM_CACHE[t_b] = build_program(t_b)
    return _PROGRAM_CACHE[t_b]


def prepare_inputs(x_emb, t_ij, edge_index, eq_w1, eq_gate_w, eq_gate_b, eq_w2,
                   ne_w1, ne_b1, ne_gate_w, ne_gate_b, ne_w2, ne_b2,
                   r_w1, r_b1, r_w2, r_b2):
    """Host-side sharding / layout. Returns (t_b, per-core input maps)."""
    x_emb = np.asarray(x_emb, np.float32)
    t_ij = np.asarray(t_ij, np.float32)
    dst = np.asarray(edge_index)[1].astype(np.int64)

    order = np.argsort(dst, kind="stable")
    dst_s = dst[order]
    t_s = np.ascontiguousarray(t_ij[order])
    cnt = np.bincount(dst, minlength=N_NODES)
    g_of = dst_s // 128                      # global 128-node block, sorted
    gcnt = np.bincount(g_of, minlength=64)
    t_b = max(1, int(-(-int(gcnt.max()) // 128)))
    e_blk = t_b * 128
    starts = np.concatenate([[0], np.cumsum(gcnt)])

    recip_full = (1.0 / np.maximum(cnt, 1)).astype(np.float32)  # [N_NODES]
    t_s *= recip_full[dst_s][:, None]        # pre-scale: scatter-sum -> mean

    tpad = np.zeros((N_CORES, SBLK * e_blk, EC), np.float32)
    dstcol = np.full((N_CORES, 128, SBLK * t_b), 255, np.float32)  # cast to bf16 below
    for g in range(64):
        c_, b_ = divmod(g, SBLK)
        ng = int(gcnt[g])
        s = int(starts[g])
        tpad[c_, b_ * e_blk:b_ * e_blk + ng] = t_s[s:s + ng]
        dcol = np.full(e_blk, 255, np.float32)
        dcol[:ng] = (dst_s[s:s + ng] - g * 128).astype(np.float32)
        dstcol[c_, :, b_ * t_b:(b_ + 1) * t_b] = dcol.reshape(t_b, 128).T

    w1h = np.concatenate([np.asarray(eq_w1, np.float32),
                          np.asarray(ne_w1, np.float32)], 0)
    w1buf = np.ascontiguousarray(
        w1h.transpose(1, 0, 2).reshape(128, K * H)).astype(BNP)
    w2h = np.concatenate([np.asarray(eq_w2, np.float32),
                          np.asarray(ne_w2, np.float32)], 0)
    w2buf = np.ascontiguousarray(
        w2h.reshape(K, 4, 128, 128).transpose(2, 0, 1, 3).reshape(128, K * H)
    ).astype(BNP)
    gwe = np.ascontiguousarray(
        np.asarray(eq_gate_w, np.float32).transpose(1, 0, 2).reshape(128, 2 * 2560)
    ).astype(BNP)
    gbe = np.ascontiguousarray(
        np.asarray(eq_gate_b, np.float32).reshape(2, 20, 128)
        .transpose(2, 0, 1).reshape(128, 40))
    gwn = np.ascontiguousarray(
        np.asarray(ne_gate_w, np.float32).transpose(1, 0, 2).reshape(128, 2 * H)
    ).astype(BNP)
    gbn = np.ascontiguousarray(
        np.asarray(ne_gate_b, np.float32).reshape(2, 4, 128)
        .transpose(2, 0, 1).reshape(128, 8))
    b1n = np.ascontiguousarray(
        np.asarray(ne_b1, np.float32).reshape(2, 4, 128)
        .transpose(2, 0, 1).reshape(128, 8))
    b2n = np.asarray(ne_b2, np.float32).astype(BNP)          # [2, 128]
    rw1 = np.asarray(r_w1, np.float32).astype(BNP)           # [128, 128]
    rb1 = np.asarray(r_b1, np.float32).reshape(128, 1)
    rb2 = np.asarray(r_b2, np.float32).reshape(4, 1)
    rw2 = np.asarray(r_w2, np.float32).astype(BNP)           # [128, 4]
    esel = np.repeat(np.eye(4, dtype=np.float32), 128, axis=1).astype(BNP)

    shared = dict(w1=w1buf, w2=w2buf, gwe=gwe, gbe=gbe, gwn=gwn, gbn=gbn,
                  b1n=b1n, b2n=b2n, rw1=rw1, rb1=rb1, rw2=rw2, rb2=rb2, esel=esel)

    in_maps = []
    for c_ in range(N_CORES):
        xtb = np.ascontiguousarray(
            x_emb[c_ * NPC:(c_ + 1) * NPC].transpose(2, 1, 0).reshape(128, M * NPC)
        ).astype(BNP)
        # t transposed: [p=edge%128, (tile, ch)] in bf16
        tt = np.ascontiguousarray(
            tpad[c_].reshape(SBLK * t_b, 128, EC).transpose(1, 0, 2)
            .reshape(128, SBLK * t_b * EC)).astype(BNP)
        m = dict(shared)
        m.update(xt=xtb, tpad=tt, dstcol=np.ascontiguousarray(dstcol[c_]))
        in_maps.append(m)
    return t_b, in_maps


def kernel(**inputs) -> np.ndarray:
    t_b, in_maps = prepare_inputs(**inputs)
    nc = _get_program(t_b)
    res = run_bass_kernel_spmd(nc, in_maps, list(range(N_CORES)))
    out = np.empty((N_NODES, M, C), np.float32)
    for c_ in range(N_CORES):
        o = res.results[c_]["out"].astype(np.float32).reshape(128, M, NPC)
        out[c_ * NPC:(c_ + 1) * NPC] = o.transpose(2, 1, 0)
    return out
